# revision 7
# baseline (speedup 1.0000x reference)
"""Depthwise causal Conv1d (B=4, S=4096, D=2048, K=4) on 8 TRN2 NeuronCores.

Sharding: 8 cores = batch(4) x sequence-halves(2); zero communication.
Each core receives a channel-major slab x_core[D, 3 + S/2] (3 history
columns: zeros at sequence start, else the previous half's tail), computes

    out[d, s] = sum_k w[d, k] * x[d, s - 3 + k] + bias[d]

as 5 accumulating PE matmuls per [128, 512] output tile (4 shifted-input
taps with per-channel-block diagonal weight matrices + a bias tap against
an all-ones rhs), then evicts PSUM -> SBUF with plain copies on
ScalarE/VectorE. Diagonal matrices are built on-chip from a 40 KB table.
"""

import numpy as np

import concourse.bacc as bacc
import concourse.bass as bass
import concourse.mybir as mybir
from concourse.bass_utils import run_bass_kernel_spmd
from concourse.masks import make_identity
from concourse.tile import TileContext

B, S, D, K = 4, 4096, 2048, 4
NCORES = 8
SHALF = S // 2          # 2048 sequence positions per core
HIST = K - 1            # 3 history columns
NBLK = D // 128         # 16 channel blocks
NT = SHALF // 512       # 4 free-dim tiles of 512 per block
NTAB = (K + 1) * NBLK   # 80 diagonal matrices: 4 weight taps + 1 bias per block
F32 = mybir.dt.float32

_CACHE = {}


def _build_program():
    if "nc" in _CACHE:
        return _CACHE["nc"]
    nc = bacc.Bacc("TRN2", num_devices=NCORES)
    x_d = nc.dram_tensor("xin", [D, SHALF + HIST], F32, kind="ExternalInput").ap()
    w_d = nc.dram_tensor("wtab", [128, NTAB], F32, kind="ExternalInput").ap()
    o_d = nc.dram_tensor("out", [D, SHALF], F32, kind="ExternalOutput").ap()

    with TileContext(nc) as tc:
        with (
            tc.tile_pool(name="const", bufs=1) as const,
            tc.tile_pool(name="xpool", bufs=3) as xpool,
            tc.tile_pool(name="opool", bufs=3) as opool,
            tc.tile_pool(name="psum", bufs=8, space="PSUM") as pp,
        ):
            wsb = const.tile([128, NTAB], F32, tag="wsb")
            nc.sync.dma_start(out=wsb[:], in_=w_d)
            mask = const.tile([128, 128], F32, tag="mask")
            make_identity(nc, mask[:])
            ones = const.tile([128, 512], F32, tag="ones")
            nc.gpsimd.memset(ones[:], 1.0)

            # DVE-local copies so every tensor_scalar below has only
            # same-engine deps (TensorScalarPtr has a single sync-wait slot).
            wsb2 = const.tile([128, NTAB], F32, tag="wsb2")
            nc.vector.tensor_copy(out=wsb2[:], in_=wsb[:])
            mask2 = const.tile([128, 128], F32, tag="mask2")
            nc.vector.tensor_copy(out=mask2[:], in_=mask[:])

            # diag[:, j*128:(j+1)*128] = diag(wtab[:, j]) built on DVE
            diag = const.tile([128, NTAB * 128], F32, tag="diag")
            for j in range(NTAB):
                nc.vector.tensor_scalar_mul(
                    diag[:, j * 128 : (j + 1) * 128], mask2[:], wsb2[:, j : j + 1]
                )

            def dslice(j):
                return diag[:, j * 128 : (j + 1) * 128]

            for blk in range(NBLK):
                xt = xpool.tile([128, SHALF + HIST], F32, tag="xt")
                nc.sync.dma_start(
                    out=xt[:], in_=x_d[blk * 128 : (blk + 1) * 128, :]
                )
                ot = opool.tile([128, SHALF], F32, tag="ot")
                ps = [
                    pp.tile([128, 512], F32, tag="ps", name=f"ps{blk}_{t}")
                    for t in range(NT)
                ]
                for k in range(K):
                    lhsT = dslice(k * NBLK + blk)
                    for t in range(NT):
                        nc.tensor.matmul(
                            ps[t][:],
                            lhsT,
                            xt[:, t * 512 + k : t * 512 + k + 512],
                            start=(k == 0),
                            stop=False,
                        )
                bias_lhsT = dslice(K * NBLK + blk)
                for t in range(NT):
                    nc.tensor.matmul(
                        ps[t][:], bias_lhsT, ones[:], start=False, stop=True
                    )
                for t in range(NT):
                    dst = ot[:, t * 512 : (t + 1) * 512]
                    if t % 2 == 0:
                        nc.scalar.copy(dst, ps[t][:])
                    else:
                        nc.vector.tensor_copy(out=dst, in_=ps[t][:])
                nc.sync.dma_start(
                    out=o_d[blk * 128 : (blk + 1) * 128, :], in_=ot[:]
                )

    nc.compile()
    _CACHE["nc"] = nc
    return nc


def _shard_inputs(x, weight, bias):
    x = np.asarray(x, dtype=np.float32)
    weight = np.asarray(weight, dtype=np.float32)
    bias = np.asarray(bias, dtype=np.float32)

    # wtab[p, k*NBLK + blk] = weight[blk*128 + p, 0, k]  for k < K
    # wtab[p, K*NBLK + blk] = bias[blk*128 + p]
    wr = weight[:, 0, :].reshape(NBLK, 128, K)          # [blk, p, k]
    wtab = np.empty((128, NTAB), dtype=np.float32)
    wtab[:, : K * NBLK] = wr.transpose(1, 2, 0).reshape(128, K * NBLK)
    wtab[:, K * NBLK :] = bias.reshape(NBLK, 128).T

    in_maps = []
    for core in range(NCORES):
        b, h = divmod(core, 2)
        s0 = h * SHALF
        xc = np.empty((D, SHALF + HIST), dtype=np.float32)
        xbt = x[b].T  # [D, S] view
        if s0 == 0:
            xc[:, :HIST] = 0.0
            xc[:, HIST:] = xbt[:, :SHALF]
        else:
            xc[:] = xbt[:, s0 - HIST : s0 + SHALF]
        in_maps.append({"xin": xc, "wtab": wtab})
    return in_maps


def _run(x, weight, bias, trace=False):
    nc = _build_program()
    in_maps = _shard_inputs(x, weight, bias)
    res = run_bass_kernel_spmd(nc, in_maps, list(range(NCORES)), trace=trace)
    out = np.empty((B, S, D), dtype=np.float32)
    for core in range(NCORES):
        b, h = divmod(core, 2)
        out[b, h * SHALF : (h + 1) * SHALF, :] = res.results[core]["out"].T
    return out, res


def kernel(x, weight, bias):
    out, _ = _run(x, weight, bias, trace=False)
    return out


# revision 23
# speedup vs baseline: 31.7093x; 31.7093x over previous
"""Depthwise causal Conv1d (B=4, S=4096, D=2048, K=4) on 8 TRN2 NeuronCores.

Sharding: 8 cores = batch(4) x sequence-halves(2); zero communication.
Each core receives a channel-major slab x_core[D, 3 + S/2] (3 history
columns: zeros at sequence start, else the previous half's tail), computes

    out[d, s] = sum_k w[d, k] * x[d, s - 3 + k] + bias[d]

with per-128-channel-block ops (free dim = 2048 sequence positions)
spread over three engines (walrus only allows per-partition-scalar ops
on DVE and ACT; POOL gets the plain tensor add):

    m3 = x3 * w3 + bias         (ACT  activation, scale+bias APs)
    m2 = x2 * w2                (ACT  activation, scale AP)
    s  = m3 + m2                (POOL tensor_tensor add)
    b  = x1 * w1 + s            (DVE  scalar_tensor_tensor)
    o  = x0 * w0 + b            (DVE  scalar_tensor_tensor -> out tile)

All DMAs are contiguous ~1 MB slabs over 128 partitions; inputs ride the
SP HWDGE ring, outputs the ACT ring, so neither blocks the other.
"""

import numpy as np

import concourse.bacc as bacc
import concourse.mybir as mybir
from concourse.bass_utils import run_bass_kernel_spmd
from concourse.tile import TileContext

B, S, D, K = 4, 4096, 2048, 4
NCORES = 8
SHALF = S // 2          # 2048 sequence positions per core
HIST = K - 1            # 3 history columns
NBLK = D // 128         # 16 channel blocks
F32 = mybir.dt.float32
MULT = mybir.AluOpType.mult
ADD = mybir.AluOpType.add

_CACHE = {}


def _build_program(nreps=1):
    key = ("nc", nreps)
    if key in _CACHE:
        return _CACHE[key]
    nc = bacc.Bacc("TRN2", num_devices=NCORES)
    x_d = nc.dram_tensor("xin", [D, SHALF + HIST], F32, kind="ExternalInput").ap()
    # wtab[p, k*NBLK+blk] = w[blk*128+p, k] for k<4; wtab[p, 4*NBLK+blk] = bias
    w_d = nc.dram_tensor("wtab", [128, (K + 1) * NBLK], F32, kind="ExternalInput").ap()
    o_d = nc.dram_tensor("out", [D, SHALF], F32, kind="ExternalOutput").ap()

    with TileContext(nc) as tc:
        with (
            tc.tile_pool(name="const", bufs=1) as const,
            tc.tile_pool(name="xpool", bufs=6) as xpool,
            tc.tile_pool(name="m3pool", bufs=3) as m3pool,
            tc.tile_pool(name="m2pool", bufs=3) as m2pool,
            tc.tile_pool(name="spool", bufs=3) as spool,
            tc.tile_pool(name="bpool", bufs=3) as bpool,
            tc.tile_pool(name="opool", bufs=5) as opool,
        ):
            wsb = const.tile([128, (K + 1) * NBLK], F32, tag="wsb")
            nc.sync.dma_start(out=wsb[:], in_=w_d)

            def wcol(k, blk):
                return wsb[:, k * NBLK + blk : k * NBLK + blk + 1]

            # out-DMAs are issued OUT_DELAY blocks late so the ACT sequencer
            # never stalls waiting for a chain result before its next
            # activation op (software-pipelined DMA issue)
            OUT_DELAY = 2
            pending = []

            def flush_out(upto):
                while pending and pending[0][0] <= upto:
                    i, tile_ap = pending.pop(0)
                    i %= NBLK
                    nc.scalar.dma_start(
                        out=o_d[i * 128 : (i + 1) * 128, :], in_=tile_ap
                    )

            for blk_r in range(NBLK * nreps):
                blk = blk_r % NBLK
                xt = xpool.tile([128, SHALF + HIST], F32, tag="xt")
                nc.sync.dma_start(
                    out=xt[:], in_=x_d[blk * 128 : (blk + 1) * 128, :]
                )
                ot = opool.tile([128, SHALF], F32, tag="ot")

                def tap(k):
                    return xt[:, k : k + SHALF]

                m3 = m3pool.tile([128, SHALF], F32, tag="m3")
                nc.scalar.activation(
                    m3[:],
                    tap(3),
                    mybir.ActivationFunctionType.Identity,
                    bias=wcol(K, blk),
                    scale=wcol(3, blk),
                )
                m2 = m2pool.tile([128, SHALF], F32, tag="m2")
                nc.scalar.activation(
                    m2[:],
                    tap(2),
                    mybir.ActivationFunctionType.Copy,
                    bias=0.0,
                    scale=wcol(2, blk),
                )
                s = spool.tile([128, SHALF], F32, tag="s")
                nc.gpsimd.tensor_tensor(out=s[:], in0=m3[:], in1=m2[:], op=ADD)
                b = bpool.tile([128, SHALF], F32, tag="b")
                nc.vector.scalar_tensor_tensor(
                    b[:], tap(1), wcol(1, blk), s[:], MULT, ADD
                )
                nc.vector.scalar_tensor_tensor(
                    ot[:], tap(0), wcol(0, blk), b[:], MULT, ADD
                )
                pending.append((blk_r, ot[:]))
                flush_out(blk_r - OUT_DELAY)
            flush_out(NBLK * nreps)

    nc.compile()
    _CACHE["nc"] = nc
    return nc


def _shard_inputs(x, weight, bias):
    x = np.asarray(x, dtype=np.float32)
    weight = np.asarray(weight, dtype=np.float32)
    bias = np.asarray(bias, dtype=np.float32)

    wr = weight[:, 0, :].reshape(NBLK, 128, K)          # [blk, p, k]
    wtab = np.empty((128, (K + 1) * NBLK), dtype=np.float32)
    wtab[:, : K * NBLK] = wr.transpose(1, 2, 0).reshape(128, K * NBLK)
    wtab[:, K * NBLK :] = bias.reshape(NBLK, 128).T

    in_maps = []
    for core in range(NCORES):
        b, h = divmod(core, 2)
        s0 = h * SHALF
        xc = np.empty((D, SHALF + HIST), dtype=np.float32)
        xbt = x[b].T  # [D, S] view
        if s0 == 0:
            xc[:, :HIST] = 0.0
            xc[:, HIST:] = xbt[:, :SHALF]
        else:
            xc[:] = xbt[:, s0 - HIST : s0 + SHALF]
        in_maps.append({"xin": xc, "wtab": wtab})
    return in_maps


def _run(x, weight, bias, trace=False):
    nc = _build_program()
    in_maps = _shard_inputs(x, weight, bias)
    res = run_bass_kernel_spmd(nc, in_maps, list(range(NCORES)), trace=trace)
    out = np.empty((B, S, D), dtype=np.float32)
    for core in range(NCORES):
        b, h = divmod(core, 2)
        out[b, h * SHALF : (h + 1) * SHALF, :] = res.results[core]["out"].T
    return out, res


def kernel(x, weight, bias):
    out, _ = _run(x, weight, bias, trace=False)
    return out
